# revision 5
# baseline (speedup 1.0000x reference)
"""BiESN2D on 8 TRN2 NeuronCores (Bass/Tile).

Reference computes 4 directional leaky-tanh ESN scans over a (8,128,128,64)
image batch: horizontal fwd/bwd over rows, vertical fwd/bwd over columns,
each with U=256 units, outputs concatenated to (8,128,128,1024).

Sharding: core = (scan-direction, batch-half).  Each of the 8 cores runs ONE
scan type over S=512 sequences (4 batches x 128 rows/cols), T=128 steps.

Device recurrence (state kept transposed, u on partitions, s on free dim):
    w_t = 0.1*w_{t-1} + tanh(x_t @ Wk + w_{t-1} @ (0.9*Wr)),   h_t = 0.9*w_t
(the 0.9 scale and final layout transposes are applied on the host).

Structure per step, per s-chunk (3 independent recurrence chains pipeline
the PE -> ScalarE(tanh) -> VectorE(blend) serial path):
  - 6 matmuls accumulate z = Wk^T x_t + Wr'^T w into ONE psum bank holding
    both 128-wide u'-tiles side by side (single accumulation group; the
    step-(t+1) K=64 x-matmuls are emitted early to fill PE stall gaps),
  - one ScalarE tanh (psum fp32 -> sbuf fp16),
  - one VectorE scalar_tensor_tensor split as tensor_scalar (off critical
    path, 4x mode) + tensor_tensor add (2x) -- the fused STT op only has a
    1x-mode uop (measured 691ns vs 333+252 split),
  - every 4 steps per chain (steps staggered across chains), one batched
    DMA of 4 ring slots to DRAM on the Sync HWDGE queue.  Input x/weight
    DMAs go on the Scalar engine's HWDGE queue instead: with everything on
    one queue the first y-DMA waits behind the whole 47us x stream and the
    ring WAR stalled the pipeline 26us at step 8.
Matmul emission zigzags the per-chain (j,k) order so the first Wr matmul of
chains 1/2 (and the heater) reuses the immediately preceding stationary and
is emitted with ldweights=False: those first matmuls carry the semaphore
wait for the chain state, and the wait otherwise lands on their LDWEIGHTS,
exposing its ~95ns on the critical path.
A dep-free N=512 "heater" matmul per step plus an initial heater burst keeps
the PE HAM clock-gate at K=8/8 (2.4 GHz); without it the PE drops to 1.2 GHz
and can latch cold (cayman HAM stuck-throttle), costing ~1.3-2x.
NOTE: all matmuls of one accumulation group must use the SAME PE row half --
mixing row groups within one group crashes the hardware.
All compute except PSUM accumulation is fp16 (measured 5.0e-4 rel l2 error
vs the fp32 reference; scaled absmax 1.9e-3).
"""

import numpy as np
from contextlib import ExitStack

import concourse.bass as bass
import concourse.mybir as mybir
import concourse.tile as tile
from concourse import bacc
from concourse.bass_utils import run_bass_kernel_spmd

# ---------------- problem constants (hardcoded per spec) ----------------
B, NH, NW, C = 8, 128, 128, 64
U = 256           # units per directional ESN cell
T = 128           # scan length
S = 512           # sequences per core (4 batches * 128)
LEAKY = 0.9
N_CORES = 8

F16 = mybir.dt.float16
F32 = mybir.dt.float32

CHUNKS = (176, 176, 160)  # s-chunks; each <= 256 (two u'-tiles in one bank)
RING = 16                 # w-state ring slots per chain
DMA_BATCH = 4             # t-steps per output DMA
XDMA_TGROUP = 16          # t-steps per input DMA chunk
HEAT_BURST = 16           # initial heater matmuls (N=512): ~7us HAM warmup
HEAT_PER_STEP = 0         # per-step heaters get hoisted into one 26us block by the scheduler -- steady-state PE density keeps HAM warm
DEDUP_LDW = True          # skip LDWEIGHTS on same-stationary repeats

# per-chain (j,k) emission orders: consecutive chains share a boundary
# stationary; heat uses (1,0) to chain with ch2's last matmul.
WR_ORDERS = [
    [(0, 0), (0, 1), (1, 0), (1, 1)],
    [(1, 1), (1, 0), (0, 1), (0, 0)],
    [(0, 0), (0, 1), (1, 1), (1, 0)],
]
HEAT_JK = (1, 0)


def build_program(chunks=CHUNKS, t_steps=T, s_total=S,
                  heat_burst=HEAT_BURST, heat_per_step=HEAT_PER_STEP):
    """Build the SPMD per-core Bass program (identical on all 8 cores)."""
    assert sum(chunks) == s_total and all(c <= 256 for c in chunks)
    assert t_steps % DMA_BATCH == 0 and RING % DMA_BATCH == 0

    nc = bacc.Bacc("TRN2", target_bir_lowering=False, debug=False,
                   num_devices=N_CORES)

    # x duplicated on both partition halves:
    #   x_d[c, t*S + s] = x_d[64 + c, t*S + s] = x[s, t, c]
    x_d = nc.declare_dram_parameter("x", [128, t_steps * s_total], F16,
                                    isOutput=False)
    # wk duplicated over both partition halves: wk2[p,:] = Wk[p%64,:]
    wk_d = nc.declare_dram_parameter("wk", [128, 256], F16, isOutput=False)
    wr_d = nc.declare_dram_parameter("wr", [256, 256], F16, isOutput=False)
    # per-chain outputs: y{ch}[p, t, j, s_local] = w_t[u = j*128 + p, s]
    y_aps = [nc.declare_dram_parameter(f"y{ch}", [128, t_steps, 2, ncs],
                                       F16, isOutput=True).ap()
             for ch, ncs in enumerate(chunks)]
    x_ap, wk_ap, wr_ap = x_d.ap(), wk_d.ap(), wr_d.ap()

    nch = len(chunks)
    offs = [sum(chunks[:i]) for i in range(nch)]
    Tanh = mybir.ActivationFunctionType.Tanh
    MUL, ADD = mybir.AluOpType.mult, mybir.AluOpType.add

    def slot(ch, t):
        return (t + ch) % RING

    with ExitStack() as ctx:
        tc = ctx.enter_context(tile.TileContext(nc))
        const = ctx.enter_context(tc.tile_pool(name="const", bufs=1))
        x_sb = const.tile([128, t_steps * s_total], F16)
        wk_sb = const.tile([128, 256], F16)
        wr0_sb = const.tile([128, 256], F16)
        wr1_sb = const.tile([128, 256], F16)
        junk = const.tile([128, 512], F16)
        # per-chain state rings: slot k at cols [k*2*ncs, (k+1)*2*ncs)
        rings = [const.tile([128, RING * 2 * chunks[ch]], F16,
                            name=f"wring{ch}") for ch in range(nch)]

        # input loads on the Scalar HWDGE queue (idle during the prologue)
        nc.scalar.dma_start(wk_sb[:], wk_ap[:])
        nc.scalar.dma_start(wr0_sb[:], wr_ap[0:128, :])
        nc.scalar.dma_start(wr1_sb[:], wr_ap[128:256, :])
        nc.vector.memset(junk[:], 0.0)
        for ch in range(nch):
            # init state = the slot step 0 reads (slot of t=-1)
            ncs = chunks[ch]
            k = slot(ch, -1)
            nc.vector.memset(rings[ch][:, k * 2 * ncs:(k + 1) * 2 * ncs], 0.0)
        for tt in range(0, t_steps, XDMA_TGROUP):
            lo, hi = tt * s_total, min(tt + XDMA_TGROUP, t_steps) * s_total
            nc.scalar.dma_start(x_sb[:, lo:hi], x_ap[:, lo:hi])

        g_pool = ctx.enter_context(tc.tile_pool(name="g", bufs=4))
        tmp_pool = ctx.enter_context(tc.tile_pool(name="tmp", bufs=3))
        ps_pool = ctx.enter_context(tc.tile_pool(name="ps", bufs=2,
                                                 space="PSUM"))
        heat_ps = ps_pool.tile([128, 512], F32, tag="heat", name="heat_ps")

        def wr_st(j, k):
            return [wr0_sb, wr1_sb][k][:, j * 128:(j + 1) * 128]

        heat_n = [0]

        def heat(n):
            for _ in range(n):
                mm = nc.tensor.matmul(heat_ps[:], wr_st(*HEAT_JK), junk[:],
                                      start=True, stop=True)
                # every heat after the first follows a matmul with the same
                # stationary (previous heat, or ch2's last Wr matmul)
                if DEDUP_LDW and heat_n[0] > 0:
                    mm.ins.ldweights = False
                heat_n[0] += 1

        def rslot(ch, t):
            ncs = chunks[ch]
            k = slot(ch, t)
            return rings[ch][:, k * 2 * ncs:(k + 1) * 2 * ncs]

        # initial heater burst: warms HAM while x streams in
        heat(heat_burst)

        def x_mms(t):
            """Open step-t accumulation groups: the two K=64 x matmuls per
            chain.  Each chain keeps ONE PE row half for its whole group
            (mixing halves within a group crashes the HW), but adjacent
            chains use different halves + psum banks, so their matmuls can
            overlap in the PE array."""
            xcol = t * s_total
            tiles = [ps_pool.tile([128, 2 * chunks[ch]], F32, tag=f"ps{ch}",
                                  name=f"ps{ch}_{t}") for ch in range(nch)]
            for j in range(2):
                for ch in range(nch):
                    ncs, off = chunks[ch], offs[ch]
                    half = 64 * (ch % 2)
                    sl = slice(xcol + off, xcol + off + ncs)
                    nc.tensor.matmul(tiles[ch][:, j * ncs:(j + 1) * ncs],
                                     wk_sb[half:half + 64,
                                           j * 128:(j + 1) * 128],
                                     x_sb[half:half + 64, sl],
                                     start=(j == 0), stop=False)
            return tiles

        def y_dma(ch, t, nsteps):
            """DMA ring slots for steps [t-nsteps+1 .. t] (contiguous)."""
            ncs = chunks[ch]
            k0 = slot(ch, t - nsteps + 1)
            assert k0 + nsteps <= RING
            src = rings[ch][:, k0 * 2 * ncs:(k0 + nsteps) * 2 * ncs]
            dst = y_aps[ch][:, t - nsteps + 1:t + 1, :, :]
            nc.sync.dma_start(dst, src)

        cur_ps = x_mms(0)
        for t in range(t_steps):
            heat(heat_per_step)
            nxt_ps = x_mms(t + 1) if t + 1 < t_steps else None
            for ch in range(nch):
                ncs, off = chunks[ch], offs[ch]
                ps, wp = cur_ps[ch], rslot(ch, t - 1)
                # 0.1*w_old depends only on the previous blend and runs on
                # the otherwise-idle GPSIMD while tanh is in flight; only
                # the cheap 2x tensor_tensor ADD stays on the DVE chain.
                tmp = tmp_pool.tile([128, 2 * ncs], F16, tag=f"tmp{ch}",
                                    name=f"tmp{ch}_{t}")
                nc.gpsimd.tensor_scalar_mul(tmp[:], wp[:], 1.0 - LEAKY)
                order = WR_ORDERS[ch]
                for i, (j, k) in enumerate(order):
                    mm = nc.tensor.matmul(
                        ps[:, j * ncs:(j + 1) * ncs], wr_st(j, k),
                        wp[:, k * ncs:(k + 1) * ncs],
                        start=False, stop=(i == len(order) - 1))
                    # chains 1/2: first matmul repeats the previous chain's
                    # boundary stationary -> its state-wait need not pay a
                    # serial LDWEIGHTS after the wait resolves
                    if DEDUP_LDW and ch > 0 and i == 0:
                        mm.ins.ldweights = False

                g = g_pool.tile([128, 2 * ncs], F16, tag=f"g{ch}",
                                name=f"g{ch}_{t}")
                nc.scalar.activation(g[:], ps[:], Tanh)
                nc.vector.tensor_add(rslot(ch, t)[:], tmp[:], g[:])

                if slot(ch, t) % DMA_BATCH == DMA_BATCH - 1:
                    y_dma(ch, t, min(DMA_BATCH, t + 1))
            cur_ps = nxt_ps
        # tail flush: chains whose last batch was partial
        for ch in range(nch):
            t_last = t_steps - 1
            rem = (slot(ch, t_last) % DMA_BATCH) + 1
            if rem != DMA_BATCH:
                y_dma(ch, t_last, rem)

    nc.compile()
    return nc


_PROGRAM = None

# test-harness knob: when trace=True, the BassKernelResults (with
# exec_time_ns from neuron-profile) is stashed in PROFILE["last"].
PROFILE = {"trace": False, "last": None}


def _get_program():
    global _PROGRAM
    if _PROGRAM is None:
        _PROGRAM = build_program()
    return _PROGRAM


def _pack_x(xs, t_steps, s_total):
    """(S, T, C) fp32 -> packed (128, T*S) fp16, duplicated on both halves."""
    xt = np.ascontiguousarray(xs.transpose(2, 1, 0))      # (C, T, S)
    packed = np.empty((128, t_steps * s_total), np.float16)
    pv = packed.reshape(2, 64, t_steps * s_total)
    pv[0] = xt.reshape(64, -1)
    pv[1] = pv[0]
    return packed


def kernel(**inputs):
    x = np.asarray(inputs["inputs"], np.float32)          # (8,128,128,64)
    wsets = [
        (np.asarray(inputs["h_fwd_k"]), np.asarray(inputs["h_fwd_r"])),
        (np.asarray(inputs["h_bwd_k"]), np.asarray(inputs["h_bwd_r"])),
        (np.asarray(inputs["v_fwd_k"]), np.asarray(inputs["v_fwd_r"])),
        (np.asarray(inputs["v_bwd_k"]), np.asarray(inputs["v_bwd_r"])),
    ]
    nc = _get_program()

    in_maps = []
    for core in range(N_CORES):
        scan, bhalf = core // 2, core % 2
        xb = x[bhalf * 4:(bhalf + 1) * 4]                 # (4, NH, NW, C)
        if scan >= 2:                                     # vertical: cols as seqs
            xb = xb.transpose(0, 2, 1, 3)                 # (4, NW, NH, C)
        xs = xb.reshape(S, T, C)
        if scan % 2 == 1:                                 # bwd: reverse time
            xs = np.ascontiguousarray(xs[:, ::-1])
        wk, wr = wsets[scan]
        wk2 = np.concatenate([wk, wk], axis=0).astype(np.float16)   # (128,256)
        wr16 = (LEAKY * wr).astype(np.float16)                      # (256,256)
        in_maps.append({"x": _pack_x(xs, T, S), "wk": wk2, "wr": wr16})

    res = run_bass_kernel_spmd(nc, in_maps, list(range(N_CORES)),
                               trace=PROFILE["trace"])
    PROFILE["last"] = res
    results = res.results

    out = np.empty((B, NH, NW, 4 * U), np.float32)
    for core in range(N_CORES):
        scan, bhalf = core // 2, core % 2
        # concat per-chain outputs (128, T, 2, ncs) back to (128, T, 2, S)
        y = np.concatenate([results[core][f"y{ch}"]
                            for ch in range(len(CHUNKS))], axis=3)
        h = LEAKY * y.astype(np.float32)
        hs = h.transpose(3, 1, 2, 0).reshape(S, T, U)     # (s, t, u=(j,p))
        if scan % 2 == 1:
            hs = hs[:, ::-1]
        dst = out[bhalf * 4:(bhalf + 1) * 4, :, :, scan * U:(scan + 1) * U]
        if scan < 2:
            dst[:] = hs.reshape(4, NH, NW, U)
        else:
            dst[:] = hs.reshape(4, NW, NH, U).transpose(0, 2, 1, 3)
    return out


# revision 7
# speedup vs baseline: 6.2415x; 6.2415x over previous
"""BiESN2D on 8 TRN2 NeuronCores (Bass/Tile).

Reference computes 4 directional leaky-tanh ESN scans over a (8,128,128,64)
image batch: horizontal fwd/bwd over rows, vertical fwd/bwd over columns,
each with U=256 units, outputs concatenated to (8,128,128,1024).

Sharding: core = (scan-direction, batch-half).  Each of the 8 cores runs ONE
scan type over S=512 sequences (4 batches x 128 rows/cols), T=128 steps.

Device recurrence (state kept transposed, u on partitions, s on free dim):
    w_t = 0.1*w_{t-1} + tanh(x_t @ Wk + w_{t-1} @ (0.9*Wr)),   h_t = 0.9*w_t
(the 0.9 scale and final layout transposes are applied on the host).

Structure per step, per s-chunk (3 independent recurrence chains pipeline
the PE -> ScalarE(tanh) -> VectorE(blend) serial path):
  - 6 matmuls accumulate z = Wk^T x_t + Wr'^T w into ONE psum bank holding
    both 128-wide u'-tiles side by side (single accumulation group; the
    step-(t+1) K=64 x-matmuls are emitted early to fill PE stall gaps),
  - one ScalarE tanh (psum fp32 -> sbuf fp16),
  - one VectorE scalar_tensor_tensor split as tensor_scalar (off critical
    path, 4x mode) + tensor_tensor add (2x) -- the fused STT op only has a
    1x-mode uop (measured 691ns vs 333+252 split),
  - every 4 steps per chain (steps staggered across chains), one batched
    DMA of 4 ring slots to DRAM on the Sync HWDGE queue.  Input x/weight
    DMAs go on the Scalar engine's HWDGE queue instead: with everything on
    one queue the first y-DMA waits behind the whole 47us x stream and the
    ring WAR stalled the pipeline 26us at step 8.
Matmul emission zigzags the per-chain (j,k) order so the first Wr matmul of
chains 1/2 (and the heater) reuses the immediately preceding stationary and
is emitted with ldweights=False: those first matmuls carry the semaphore
wait for the chain state, and the wait otherwise lands on their LDWEIGHTS,
exposing its ~95ns on the critical path.
A dep-free N=512 "heater" matmul per step plus an initial heater burst keeps
the PE HAM clock-gate at K=8/8 (2.4 GHz); without it the PE drops to 1.2 GHz
and can latch cold (cayman HAM stuck-throttle), costing ~1.3-2x.
NOTE: all matmuls of one accumulation group must use the SAME PE row half --
mixing row groups within one group crashes the hardware.
All compute except PSUM accumulation is fp16 (measured 5.0e-4 rel l2 error
vs the fp32 reference; scaled absmax 1.9e-3).
"""

import numpy as np
from contextlib import ExitStack

import concourse.bass as bass
import concourse.mybir as mybir
import concourse.tile as tile
from concourse import bacc
from concourse.bass_utils import run_bass_kernel_spmd

# ---------------- problem constants (hardcoded per spec) ----------------
B, NH, NW, C = 8, 128, 128, 64
U = 256           # units per directional ESN cell
T = 128           # scan length
S = 512           # sequences per core (4 batches * 128)
LEAKY = 0.9
N_CORES = 8

F16 = mybir.dt.float16
F32 = mybir.dt.float32

CHUNKS = (176, 176, 160)  # s-chunks; each <= 256 (two u'-tiles in one bank)
RING = 16                 # w-state ring slots per chain
DMA_BATCH = 4             # t-steps per output DMA
XDMA_TGROUP = 16          # t-steps per input DMA chunk
HEAT_BURST = 16           # initial heater matmuls (N=512): ~7us HAM warmup
HEAT_PER_STEP = 0         # per-step heaters get hoisted into one 26us block by the scheduler -- steady-state PE density keeps HAM warm
DEDUP_LDW = True          # skip LDWEIGHTS on same-stationary repeats

# per-chain (j,k) emission orders: consecutive chains share a boundary
# stationary; heat uses (1,0) to chain with ch2's last matmul.
WR_ORDERS = [
    [(0, 0), (0, 1), (1, 0), (1, 1)],
    [(1, 1), (1, 0), (0, 1), (0, 0)],
    [(0, 0), (0, 1), (1, 1), (1, 0)],
]
HEAT_JK = (1, 0)


def build_program(chunks=CHUNKS, t_steps=T, s_total=S,
                  heat_burst=HEAT_BURST, heat_per_step=HEAT_PER_STEP):
    """Build the SPMD per-core Bass program (identical on all 8 cores)."""
    assert sum(chunks) == s_total and all(c <= 256 for c in chunks)
    assert t_steps % DMA_BATCH == 0 and RING % DMA_BATCH == 0

    nc = bacc.Bacc("TRN2", target_bir_lowering=False, debug=False,
                   num_devices=N_CORES)

    # x duplicated on both partition halves:
    #   x_d[c, t*S + s] = x_d[64 + c, t*S + s] = x[s, t, c]
    x_d = nc.declare_dram_parameter("x", [128, t_steps * s_total], F16,
                                    isOutput=False)
    # wk duplicated over both partition halves: wk2[p,:] = Wk[p%64,:]
    wk_d = nc.declare_dram_parameter("wk", [128, 256], F16, isOutput=False)
    wr_d = nc.declare_dram_parameter("wr", [256, 256], F16, isOutput=False)
    # per-chain outputs: y{ch}[p, t, j, s_local] = w_t[u = j*128 + p, s]
    y_aps = [nc.declare_dram_parameter(f"y{ch}", [128, t_steps, 2, ncs],
                                       F16, isOutput=True).ap()
             for ch, ncs in enumerate(chunks)]
    x_ap, wk_ap, wr_ap = x_d.ap(), wk_d.ap(), wr_d.ap()

    nch = len(chunks)
    offs = [sum(chunks[:i]) for i in range(nch)]
    Tanh = mybir.ActivationFunctionType.Tanh
    MUL, ADD = mybir.AluOpType.mult, mybir.AluOpType.add

    def slot(ch, t):
        return (t + ch) % RING

    with ExitStack() as ctx:
        tc = ctx.enter_context(tile.TileContext(nc))
        const = ctx.enter_context(tc.tile_pool(name="const", bufs=1))
        x_sb = const.tile([128, t_steps * s_total], F16)
        wk_sb = const.tile([128, 256], F16)
        wr0_sb = const.tile([128, 256], F16)
        wr1_sb = const.tile([128, 256], F16)
        junk = const.tile([128, 512], F16)
        # per-chain state rings: slot k at cols [k*2*ncs, (k+1)*2*ncs)
        rings = [const.tile([128, RING * 2 * chunks[ch]], F16,
                            name=f"wring{ch}") for ch in range(nch)]

        # input loads on the Scalar HWDGE queue (idle during the prologue)
        nc.scalar.dma_start(wk_sb[:], wk_ap[:])
        nc.scalar.dma_start(wr0_sb[:], wr_ap[0:128, :])
        nc.scalar.dma_start(wr1_sb[:], wr_ap[128:256, :])
        nc.vector.memset(junk[:], 0.0)
        for ch in range(nch):
            # init state = the slot step 0 reads (slot of t=-1)
            ncs = chunks[ch]
            k = slot(ch, -1)
            nc.vector.memset(rings[ch][:, k * 2 * ncs:(k + 1) * 2 * ncs], 0.0)
        for tt in range(0, t_steps, XDMA_TGROUP):
            lo, hi = tt * s_total, min(tt + XDMA_TGROUP, t_steps) * s_total
            nc.scalar.dma_start(x_sb[:, lo:hi], x_ap[:, lo:hi])

        g_pool = ctx.enter_context(tc.tile_pool(name="g", bufs=4))
        tmp_pool = ctx.enter_context(tc.tile_pool(name="tmp", bufs=3))
        ps_pool = ctx.enter_context(tc.tile_pool(name="ps", bufs=2,
                                                 space="PSUM"))
        heat_ps = ps_pool.tile([128, 512], F32, tag="heat", name="heat_ps")

        def wr_st(j, k):
            return [wr0_sb, wr1_sb][k][:, j * 128:(j + 1) * 128]

        heat_n = [0]

        def heat(n):
            for _ in range(n):
                mm = nc.tensor.matmul(heat_ps[:], wr_st(*HEAT_JK), junk[:],
                                      start=True, stop=True)
                # every heat after the first follows a matmul with the same
                # stationary (previous heat, or ch2's last Wr matmul)
                if DEDUP_LDW and heat_n[0] > 0:
                    mm.ins.ldweights = False
                heat_n[0] += 1

        def rslot(ch, t):
            ncs = chunks[ch]
            k = slot(ch, t)
            return rings[ch][:, k * 2 * ncs:(k + 1) * 2 * ncs]

        # initial heater burst: warms HAM while x streams in
        heat(heat_burst)

        def x_mms(t):
            """Open step-t accumulation groups: the two K=64 x matmuls per
            chain.  Each chain keeps ONE PE row half for its whole group
            (mixing halves within a group crashes the HW), but adjacent
            chains use different halves + psum banks, so their matmuls can
            overlap in the PE array."""
            xcol = t * s_total
            tiles = [ps_pool.tile([128, 2 * chunks[ch]], F32, tag=f"ps{ch}",
                                  name=f"ps{ch}_{t}") for ch in range(nch)]
            for j in range(2):
                for ch in range(nch):
                    ncs, off = chunks[ch], offs[ch]
                    half = 64 * (ch % 2)
                    sl = slice(xcol + off, xcol + off + ncs)
                    nc.tensor.matmul(tiles[ch][:, j * ncs:(j + 1) * ncs],
                                     wk_sb[half:half + 64,
                                           j * 128:(j + 1) * 128],
                                     x_sb[half:half + 64, sl],
                                     start=(j == 0), stop=False)
            return tiles

        def y_dma(ch, t, nsteps):
            """DMA ring slots for steps [t-nsteps+1 .. t] (contiguous)."""
            ncs = chunks[ch]
            k0 = slot(ch, t - nsteps + 1)
            assert k0 + nsteps <= RING
            src = rings[ch][:, k0 * 2 * ncs:(k0 + nsteps) * 2 * ncs]
            dst = y_aps[ch][:, t - nsteps + 1:t + 1, :, :]
            nc.sync.dma_start(dst, src)

        cur_ps = x_mms(0)
        for t in range(t_steps):
            heat(heat_per_step)
            nxt_ps = None
            for ch in range(nch):
                ncs, off = chunks[ch], offs[ch]
                ps, wp = cur_ps[ch], rslot(ch, t - 1)
                # 0.1*w_old depends only on the previous blend, so it runs
                # on DVE (4x tensor_scalar mode) while tanh is in flight
                # (GPSIMD measured 5.2us/op for this -- unusable); only the
                # cheap 2x tensor_tensor ADD stays on the chain.
                tmp = tmp_pool.tile([128, 2 * ncs], F16, tag=f"tmp{ch}",
                                    name=f"tmp{ch}_{t}")
                nc.vector.tensor_scalar_mul(tmp[:], wp[:], 1.0 - LEAKY)
                order = WR_ORDERS[ch]
                for i, (j, k) in enumerate(order):
                    mm = nc.tensor.matmul(
                        ps[:, j * ncs:(j + 1) * ncs], wr_st(j, k),
                        wp[:, k * ncs:(k + 1) * ncs],
                        start=False, stop=(i == len(order) - 1))
                    # chains 1/2: first matmul repeats the previous chain's
                    # boundary stationary -> its state-wait need not pay a
                    # serial LDWEIGHTS after the wait resolves
                    if DEDUP_LDW and ch > 0 and i == 0:
                        mm.ins.ldweights = False

                g = g_pool.tile([128, 2 * ncs], F16, tag=f"g{ch}",
                                name=f"g{ch}_{t}")
                nc.scalar.activation(g[:], ps[:], Tanh)
                nc.vector.tensor_add(rslot(ch, t)[:], tmp[:], g[:])

                if slot(ch, t) % DMA_BATCH == DMA_BATCH - 1:
                    y_dma(ch, t, min(DMA_BATCH, t + 1))
                if ch == 0:
                    # open step-(t+1) groups after chain 0's block: the
                    # t+1 psum banks were freed by tanh(t-1, *), which is
                    # ancient by this point in program order, so these
                    # matmuls carry no semaphore wait and never pay a
                    # serialized post-wait LDWEIGHTS; the block still sits
                    # ahead of chains 1/2's state-wait stalls as filler.
                    nxt_ps = x_mms(t + 1) if t + 1 < t_steps else None
            cur_ps = nxt_ps
        # tail flush: chains whose last batch was partial
        for ch in range(nch):
            t_last = t_steps - 1
            rem = (slot(ch, t_last) % DMA_BATCH) + 1
            if rem != DMA_BATCH:
                y_dma(ch, t_last, rem)

    nc.compile()
    return nc


_PROGRAM = None

# test-harness knob: when trace=True, the BassKernelResults (with
# exec_time_ns from neuron-profile) is stashed in PROFILE["last"].
PROFILE = {"trace": False, "last": None}


def _get_program():
    global _PROGRAM
    if _PROGRAM is None:
        _PROGRAM = build_program()
    return _PROGRAM


def _pack_x(xs, t_steps, s_total):
    """(S, T, C) fp32 -> packed (128, T*S) fp16, duplicated on both halves."""
    xt = np.ascontiguousarray(xs.transpose(2, 1, 0))      # (C, T, S)
    packed = np.empty((128, t_steps * s_total), np.float16)
    pv = packed.reshape(2, 64, t_steps * s_total)
    pv[0] = xt.reshape(64, -1)
    pv[1] = pv[0]
    return packed


def kernel(**inputs):
    x = np.asarray(inputs["inputs"], np.float32)          # (8,128,128,64)
    wsets = [
        (np.asarray(inputs["h_fwd_k"]), np.asarray(inputs["h_fwd_r"])),
        (np.asarray(inputs["h_bwd_k"]), np.asarray(inputs["h_bwd_r"])),
        (np.asarray(inputs["v_fwd_k"]), np.asarray(inputs["v_fwd_r"])),
        (np.asarray(inputs["v_bwd_k"]), np.asarray(inputs["v_bwd_r"])),
    ]
    nc = _get_program()

    in_maps = []
    for core in range(N_CORES):
        scan, bhalf = core // 2, core % 2
        xb = x[bhalf * 4:(bhalf + 1) * 4]                 # (4, NH, NW, C)
        if scan >= 2:                                     # vertical: cols as seqs
            xb = xb.transpose(0, 2, 1, 3)                 # (4, NW, NH, C)
        xs = xb.reshape(S, T, C)
        if scan % 2 == 1:                                 # bwd: reverse time
            xs = np.ascontiguousarray(xs[:, ::-1])
        wk, wr = wsets[scan]
        wk2 = np.concatenate([wk, wk], axis=0).astype(np.float16)   # (128,256)
        wr16 = (LEAKY * wr).astype(np.float16)                      # (256,256)
        in_maps.append({"x": _pack_x(xs, T, S), "wk": wk2, "wr": wr16})

    res = run_bass_kernel_spmd(nc, in_maps, list(range(N_CORES)),
                               trace=PROFILE["trace"])
    PROFILE["last"] = res
    results = res.results

    out = np.empty((B, NH, NW, 4 * U), np.float32)
    for core in range(N_CORES):
        scan, bhalf = core // 2, core % 2
        # concat per-chain outputs (128, T, 2, ncs) back to (128, T, 2, S)
        y = np.concatenate([results[core][f"y{ch}"]
                            for ch in range(len(CHUNKS))], axis=3)
        h = LEAKY * y.astype(np.float32)
        hs = h.transpose(3, 1, 2, 0).reshape(S, T, U)     # (s, t, u=(j,p))
        if scan % 2 == 1:
            hs = hs[:, ::-1]
        dst = out[bhalf * 4:(bhalf + 1) * 4, :, :, scan * U:(scan + 1) * U]
        if scan < 2:
            dst[:] = hs.reshape(4, NH, NW, U)
        else:
            dst[:] = hs.reshape(4, NW, NH, U).transpose(0, 2, 1, 3)
    return out
